# revision 1
# baseline (speedup 1.0000x reference)
"""GQA attention kernel for 8 Trainium2 NeuronCores (tensor-parallel over heads).

Self-contained: hardcodes shapes from the problem spec.
  x  [2, 1024, 4096]  Wq [4096, 4096]  Wk/Wv [4096, 1024]  Wo [4096, 4096]  bo [4096]
  32 q heads, 8 KV groups, head dim 128, RoPE theta 10000, causal softmax.

Sharding: core c owns KV group c and query heads 4c..4c+3.
  - xT fed replicated (no input collective); Wq/Wk/Wv column-sharded per
    head group (RoPE-permuted, scale folded into Wq).
  - QKV projection keeps q/k (plus a half-rotated copy for RoPE) and vT
    resident in SBUF; attention runs per (batch, local head).
  - Wo is ROW-sharded: each core projects its own heads' attention output
    into a full-width partial [1024, 4096] per batch, then a ReduceScatter
    per batch sums partials and hands core c rows c*128..c*128+128.
  - Host reassembles rows and adds bo.
"""

import numpy as np
import ml_dtypes

import concourse.bass as bass
import concourse.bass_isa as bass_isa
import concourse.mybir as mybir
import concourse.tile as tile
from concourse import bacc
from concourse import bass_utils

N_CORES = 8
B, T, C = 2, 1024, 4096
H, G, D = 32, 8, 128
REP = H // G             # q heads per KV group/core = 4
BT = B * T               # 2048
HD_SHARD = REP * D       # 512 q columns per core
ROPE_THETA = 10000.0
NCH = T // 128           # 8 s/t blocks per batch
OUT_ROWS = BT // N_CORES  # 256 output rows per core (128 per batch)
# per-batch ReduceScatter chunking: (row_start, row_len) of the partial
RS_CHUNKS = {0: [(0, 512), (512, 512)], 1: [(0, 512), (512, 512)]}

F32 = mybir.dt.float32
F16 = mybir.dt.float16

_CACHE = {}

# packed causal probs: row-block j holds cols t in [j*128, T)
_POFF = [0] * NCH
for _j in range(1, NCH):
    _POFF[_j] = _POFF[_j - 1] + (T - (_j - 1) * 128)
_PTOT = _POFF[-1] + (T - (NCH - 1) * 128)  # 4608


def _balanced_chunks(total, maxc=512, quantum=128):
    nblk = total // quantum
    n = -(-total // maxc)
    sizes = []
    for i in range(n):
        take = nblk // n + (1 if i < nblk % n else 0)
        sizes.append(take * quantum)
    return sizes


def _build_nc():
    nc = bacc.Bacc("TRN2", target_bir_lowering=False, debug=False, num_devices=N_CORES)

    # ---- I/O ----
    xT = nc.dram_tensor("xT", [C, BT], F16, kind="ExternalInput")
    # weights pre-shuffled on host to [p][chunk][ko][m] so DMA runs are 8KB
    wq = nc.dram_tensor("wq", [128, REP * (C // 128) * 128], F16,
                        kind="ExternalInput")
    wk = nc.dram_tensor("wk", [128, (C // 128) * D], F16, kind="ExternalInput")
    wv = nc.dram_tensor("wv", [128, (C // 128) * D], F16, kind="ExternalInput")
    wo = nc.dram_tensor("wo", [HD_SHARD, C], F16, kind="ExternalInput")
    cos2 = nc.dram_tensor("cos2", [D, T], F16, kind="ExternalInput")
    sinpm = nc.dram_tensor("sinpm", [D, T], F16, kind="ExternalInput")
    tri = nc.dram_tensor("tri", [128, 128], F16, kind="ExternalInput")
    idin = nc.dram_tensor("idin", [128, 128], F16, kind="ExternalInput")
    out = nc.dram_tensor("out", [OUT_ROWS, C], F16, kind="ExternalOutput")

    # ---- DRAM scratch ----
    partial = [
        nc.dram_tensor(f"partial_{b}", [T, C], F16, kind="Internal")
        for b in range(B)
    ]
    rsout = [
        nc.dram_tensor(f"rsout_{b}", [T // N_CORES, C], F16, kind="Internal")
        for b in range(B)
    ]

    M_ALL = HD_SHARD + 2 * D   # 768
    KSUB = C // 128            # 32  (contraction chunks)
    NT = 512                   # stage-1 t-cols per chunk
    NNT = BT // NT             # 4
    # m-chunk order: two passes per n-chunk; k & v first so attention can start
    M_GROUPS = [[4, 5, 0], [1, 2, 3]]

    xT_r = xT[:].rearrange("(ko p) t -> p ko t", p=128)
    wq_r = wq[:].rearrange("p (h ko m) -> p h ko m", h=REP, ko=KSUB)
    wk_r = wk[:].rearrange("p (ko m) -> p ko m", ko=KSUB)
    wv_r = wv[:].rearrange("p (ko m) -> p ko m", ko=KSUB)
    wo_r = wo[:].rearrange("(h p) co -> p h co", p=128)

    from contextlib import ExitStack
    with tile.TileContext(nc) as tc:
        with ExitStack() as _stk:
            _p = lambda **kw: _stk.enter_context(tc.tile_pool(**kw))
            consts = _p(name="consts", bufs=1)
            persist = _p(name="persist", bufs=1)
            s2_probs = _p(name="s2_probs", bufs=2)
            s2_attnT = _p(name="s2_attnT", bufs=8)
            s2_kv = _p(name="s2_kv", bufs=2)
            s2_q = _p(name="s2_q", bufs=2)
            s2_tmp = _p(name="s2_tmp", bufs=2)
            s2_misc = _p(name="s2_misc", bufs=2)
            s2_dseg = _p(name="s2_dseg", bufs=2)
            s2_ps_sc = _p(name="s2_ps_sc", bufs=2, space="PSUM")
            s2_ps_pv = _p(name="s2_ps_pv", bufs=2, space="PSUM")
            s2_ps_tr = _p(name="s2_ps_tr", bufs=1, space="PSUM")
            # ---------- constants (loads emitted after first weights) ----------
            ident = consts.tile([128, 128], F16)
            tri_sb = consts.tile([128, 128], F16)
            cos_sb = consts.tile([128, T], F16)
            sin_sb = consts.tile([128, T], F16)

            # ---------- persistent stage-1 outputs ----------
            qkT = persist.tile([128, REP + 1, BT], F16)      # q0..q3, k
            qkTs = persist.tile([128, REP + 1, BT], F16)     # half-rotated copy
            vT = persist.tile([128, BT], F16)
            wo_sb = persist.tile([128, REP, C], F16)

            # ================= Stage 1: QKV projection =================
            with ExitStack() as _stk1:
                _p1 = lambda **kw: _stk1.enter_context(tc.tile_pool(**kw))
                s1_w = _p1(name="s1_w", bufs=1)
                s1_x = _p1(name="s1_x", bufs=2)
                s1_psum = _p1(name="s1_psum", bufs=3, space="PSUM")
                # [p, m-chunk(6), ko, 128]: q0..q3, k, v
                w_sb = s1_w.tile([128, 6, KSUB, 128], F16)
                nc.scalar.dma_start(w_sb[:, 4], wk_r)
                nc.scalar.dma_start(w_sb[:, 5], wv_r)
                nc.scalar.dma_start(w_sb[:, 0], wq_r[:, 0])

                def _late_weights(kq):
                    if kq in (1, 3, 5):
                        hh = (kq + 1) // 2
                        nc.scalar.dma_start(w_sb[:, hh], wq_r[:, hh])
                    elif kq == 7:
                        # consts aren't needed until RoPE (~170us in)
                        nc.scalar.dma_start(ident[:], idin[:])
                        nc.scalar.dma_start(tri_sb[:], tri[:])
                        nc.scalar.dma_start(cos_sb[:], cos2[:])
                        nc.scalar.dma_start(sin_sb[:], sinpm[:])

                # PE warmup: dummy matmuls on memset tiles while the first
                # weight/x DMAs are in flight, so the HAM clock gate opens
                # (1.2 -> 2.4 GHz) before real work arrives
                wu_st = s1_w.tile([128, 128], F16, name="wu_st", tag="wu_st")
                nc.vector.memset(wu_st[:], 0.0)
                wu_mv = s1_w.tile([128, 512], F16, name="wu_mv", tag="wu_mv")
                nc.vector.memset(wu_mv[:], 0.0)
                wu_ps = s1_psum.tile([128, NT], F32, name="wu_ps", tag="s1ps")
                for _wu in range(12):
                    nc.tensor.matmul(
                        wu_ps[:], wu_st[:], wu_mv[:],
                        start=(_wu == 0), stop=(_wu == 11),
                    )

                first_grp_done = False
                for n in range(NNT):
                    nsl = slice(n * NT, (n + 1) * NT)
                    for grp in M_GROUPS:
                        psums = {
                            m: s1_psum.tile([128, NT], F32,
                                            name=f"s1ps_{m}", tag="s1ps")
                            for m in grp
                        }
                        for kq in range(KSUB // 4):  # 8 loads of 4 k-slices
                            x_sb = s1_x.tile([128, 4, NT], F16, tag="s1x")
                            nc.sync.dma_start(
                                x_sb[:], xT_r[:, kq * 4:(kq + 1) * 4, nsl]
                            )
                            if not first_grp_done:
                                _late_weights(kq)
                            for m in grp:
                                for k in range(4):
                                    nc.tensor.matmul(
                                        psums[m][:],
                                        w_sb[:, m, kq * 4 + k, :],
                                        x_sb[:, k, :],
                                        start=(kq == 0 and k == 0),
                                        stop=(kq == KSUB // 4 - 1 and k == 3),
                                    )
                        first_grp_done = True
                        for m in grp:
                            if m == 5:
                                nc.vector.tensor_copy(vT[:, nsl], psums[m][:])
                            else:
                                nc.vector.tensor_copy(qkT[:, m, nsl], psums[m][:])
                                # half-rotated copy for RoPE (SBUF->SBUF DMA,
                                # ACT queue to keep x prefetch flowing on SP)
                                nc.scalar.dma_start(
                                    qkTs[0:64, m, nsl], qkT[64:128, m, nsl])
                                nc.scalar.dma_start(
                                    qkTs[64:128, m, nsl], qkT[0:64, m, nsl])
                if True:
                    nc.scalar.dma_start(wo_sb[:], wo_r)

            # ================= Stage 2 + 3 per batch =================
            with ExitStack() as _stk3:
                _p3 = lambda **kw: _stk3.enter_context(tc.tile_pool(**kw))
                s3_ev = _p3(name="s3_ev", bufs=3)
                s3_psum = _p3(name="s3_psum", bufs=2, space="PSUM")

                def rope(dst, m, tcol):
                    tmp = s2_tmp.tile([128, T], F16, tag="rope_tmp")
                    nc.vector.tensor_tensor(
                        tmp[:], qkTs[:, m, tcol], sin_sb[:], mybir.AluOpType.mult)
                    nc.vector.tensor_tensor(
                        dst[:], qkT[:, m, tcol], cos_sb[:], mybir.AluOpType.mult)
                    nc.vector.tensor_tensor(
                        dst[:], dst[:], tmp[:], mybir.AluOpType.add)

                pending_rs = []

                def _flush_rs(pb):
                    for (cs, cl) in RS_CHUNKS[pb]:
                        ro = cs // N_CORES
                        rl = cl // N_CORES
                        nc.gpsimd.collective_compute(
                            "ReduceScatter",
                            mybir.AluOpType.add,
                            replica_groups=[list(range(N_CORES))],
                            ins=[partial[pb][cs:cs + cl, :].opt()],
                            outs=[rsout[pb][ro:ro + rl, :].opt()],
                        )
                        nc.scalar.dma_start(
                            out[pb * (T // N_CORES) + ro:
                                pb * (T // N_CORES) + ro + rl, :],
                            rsout[pb][ro:ro + rl, :],
                        )

                for b in range(B):
                    tcol = slice(b * T, (b + 1) * T)
                    k_rope = s2_kv.tile([128, T], F16, tag="k_rope")
                    rope(k_rope, REP, tcol)
                    # v: transpose vT blocks to [s, d]
                    v_sb = s2_kv.tile([128, NCH, D], F16, tag="v_sb")
                    for j in range(NCH):
                        ps_tr = s2_ps_tr.tile([128, 128], F16, tag="ps_tr")
                        nc.tensor.transpose(
                            ps_tr[:], vT[:, b * T + j * 128:b * T + (j + 1) * 128],
                            ident[:])
                        nc.vector.tensor_copy(v_sb[:, j, :], ps_tr[:])

                    attnTs = []
                    for h in range(REP):
                        q_rope = s2_q.tile([128, T], F16, tag="q_rope")
                        rope(q_rope, h, tcol)

                        # scoresT[s, t] = k_rope.T @ q_rope -> exp -> probs
                        probs = s2_probs.tile([128, _PTOT], F16, tag="probs")
                        acc = s2_misc.tile([1, T], F16, tag="acc")
                        for j in range(NCH):
                            t0 = j * 128
                            seg = T - t0
                            off = t0
                            for w in _balanced_chunks(seg):
                                ps_sc = s2_ps_sc.tile([128, 512], F32, tag="ps_sc")
                                nc.tensor.matmul(
                                    ps_sc[:, 0:w],
                                    k_rope[:, t0:t0 + 128],
                                    q_rope[:, off:off + w],
                                    start=True,
                                    stop=True,
                                )
                                nc.scalar.activation(
                                    probs[:, _POFF[j] + off - t0:
                                          _POFF[j] + off - t0 + w],
                                    ps_sc[:, 0:w],
                                    mybir.ActivationFunctionType.Exp,
                                )
                                off += w
                            # mask on gpsimd: feeds the partition reduce on
                            # the same queue (no cross-engine sem hop)
                            nc.gpsimd.tensor_tensor(
                                probs[:, _POFF[j]:_POFF[j] + 128],
                                probs[:, _POFF[j]:_POFF[j] + 128],
                                tri_sb[:],
                                mybir.AluOpType.mult,
                            )
                            # denominator: partial column-sums of this segment
                            dseg = s2_dseg.tile([128, T], F16, tag="dseg")
                            nc.gpsimd.partition_all_reduce(
                                dseg[:, 0:seg],
                                probs[:, _POFF[j]:_POFF[j] + seg],
                                channels=128,
                                reduce_op=bass_isa.ReduceOp.add,
                            )
                            with nc.allow_low_precision(
                                    reason="f16 softmax denom, 0.1% rel"):
                                if j == 0:
                                    nc.vector.tensor_copy(
                                        acc[:, :], dseg[0:1, 0:seg])
                                else:
                                    nc.vector.tensor_tensor(
                                        acc[:, t0:T], acc[:, t0:T],
                                        dseg[0:1, 0:seg],
                                        mybir.AluOpType.add)
                        rcp1 = s2_misc.tile([1, T], F16, tag="rcp1")
                        with nc.allow_low_precision(
                                reason="f16 softmax denom, 0.1% rel"):
                            nc.vector.reciprocal(rcp1[:], acc[:])
                        recipB = s2_misc.tile([128, T], F16, tag="recipB")
                        nc.gpsimd.partition_broadcast(recipB[:], rcp1[:])

                        # PV: attnT[d, t] = sum_j v_j.T @ probs_j, then
                        # normalize by the softmax denominator on eviction
                        attnT = s2_attnT.tile([128, NCH, 128], F16, tag="attnT")
                        attnTs.append(attnT)
                        NR = 512
                        for r in range(T // NR):
                            ps_pv = s2_ps_pv.tile([128, NR], F32, tag="ps_pv")
                            jmax = (r * NR) // 128 + NR // 128 - 1
                            for j in range(jmax + 1):
                                a = max(0, j * 128 - r * NR)
                                w = NR - a
                                tg = r * NR + a  # global t of slice start
                                nc.tensor.matmul(
                                    ps_pv[:, a:NR],
                                    v_sb[:, j, :],
                                    probs[:, _POFF[j] + tg - j * 128:
                                          _POFF[j] + tg - j * 128 + w],
                                    start=(j == 0),
                                    stop=(j == jmax),
                                )
                            nc.vector.tensor_tensor(
                                attnT[:, r * NR // 128:(r + 1) * NR // 128, :],
                                ps_pv[:],
                                recipB[:, r * NR:(r + 1) * NR],
                                mybir.AluOpType.mult,
                            )

                    # ---- Stage 3: partial = attn_local @ Wo_local ----
                    for i in range(NCH):
                        for cc2 in range(C // 1024):
                            ev3 = s3_ev.tile([128, 1024], F16, tag="s3ev")
                            for sub in range(2):
                                cch = cc2 * 2 + sub
                                ps3 = s3_psum.tile([128, 512], F32, tag="s3ps")
                                for h in range(REP):
                                    nc.tensor.matmul(
                                        ps3[:],
                                        attnTs[h][:, i, :],
                                        wo_sb[:, h, cch * 512:(cch + 1) * 512],
                                        start=(h == 0),
                                        stop=(h == REP - 1),
                                    )
                                nc.vector.tensor_copy(
                                    ev3[:, sub * 512:(sub + 1) * 512], ps3[:])
                            nc.sync.dma_start(
                                partial[b][i * 128:(i + 1) * 128,
                                           cc2 * 1024:(cc2 + 1) * 1024],
                                ev3[:],
                            )

                    # ---- ReduceScatter emission is deferred so its queue
                    # wait never blocks later gpsimd work (see _flush_rs) ----
                    pending_rs.append(b)
                    if b == B - 1:
                        for pb in pending_rs:
                            _flush_rs(pb)
                        pending_rs.clear()

    nc.compile()
    return nc


def _host_prep(x, Wq, Wk, Wv, Wo, bo):
    x = np.asarray(x, dtype=np.float32)
    Wq = np.asarray(Wq, dtype=np.float32)
    Wk = np.asarray(Wk, dtype=np.float32)
    Wv = np.asarray(Wv, dtype=np.float32)
    Wo = np.asarray(Wo, dtype=np.float32)

    xT = np.ascontiguousarray(x.reshape(BT, C).astype(np.float16).T)

    scale = np.float32(D ** -0.5)
    # rope-permute ([evens | odds] within each head) via reshape+transpose
    Wqp = np.ascontiguousarray(
        (Wq * scale).astype(np.float16)
        .reshape(C, H, D // 2, 2).transpose(0, 1, 3, 2).reshape(C, H, D))
    Wkp = np.ascontiguousarray(
        Wk.astype(np.float16)
        .reshape(C, G, D // 2, 2).transpose(0, 1, 3, 2).reshape(C, G, D))
    Wv16 = Wv.astype(np.float16).reshape(C, G, D)
    Wo16 = Wo.astype(np.float16)

    freqs = 1.0 / (ROPE_THETA ** (np.arange(0, D, 2, dtype=np.float64) / D))
    angle = np.arange(T, dtype=np.float64)[:, None] * freqs[None, :]  # [T, 64]
    cosh = np.cos(angle).T.astype(np.float16)   # [64, T]
    sinh = np.sin(angle).T.astype(np.float16)
    cos2 = np.ascontiguousarray(np.vstack([cosh, cosh]))       # [128, T]
    sinpm = np.ascontiguousarray(np.vstack([-sinh, sinh]))     # [128, T]

    sidx = np.arange(128)[:, None]
    tidx = np.arange(128)[None, :]
    tri = np.ascontiguousarray((sidx <= tidx).astype(np.float16))
    ident = np.eye(128, dtype=np.float16)

    def _pshuf(W):
        # [C, M] -> [p, M/128 chunks? no: [p][ko][m]] flattened per partition
        KO = C // 128
        M = W.shape[1]
        return np.ascontiguousarray(
            W.reshape(KO, 128, M).transpose(1, 0, 2).reshape(128, KO * M))

    in_maps = []
    for c in range(N_CORES):
        Wqc = Wqp[:, c * REP:(c + 1) * REP]  # [C, REP, D]
        # [p][h][ko][m]
        wq_host = np.ascontiguousarray(
            Wqc.reshape(C // 128, 128, REP, D)
            .transpose(1, 2, 0, 3).reshape(128, REP * (C // 128) * D))
        in_maps.append({
            "xT": xT,
            "wq": wq_host,
            "wk": _pshuf(Wkp[:, c]),
            "wv": _pshuf(Wv16[:, c]),
            "wo": np.ascontiguousarray(Wo16[c * HD_SHARD:(c + 1) * HD_SHARD, :]),
            "cos2": cos2,
            "sinpm": sinpm,
            "tri": tri,
            "idin": ident,
        })
    return in_maps


def _run(x, Wq, Wk, Wv, Wo, bo, trace=False, trace_cores=None):
    in_maps = _host_prep(x, Wq, Wk, Wv, Wo, bo)
    if "nc" not in _CACHE:
        _CACHE["nc"] = _build_nc()
    nc = _CACHE["nc"]
    r = bass_utils.run_bass_kernel_spmd(
        nc, in_maps, core_ids=list(range(N_CORES)),
        trace=trace, trace_cores=trace_cores,
    )
    RPB = T // N_CORES  # rows per (core, batch)
    out = np.empty((BT, C), dtype=np.float32)
    for c in range(N_CORES):
        res = r.results[c]["out"]
        for b in range(B):
            for (cs, cl) in RS_CHUNKS[b]:
                ro = cs // N_CORES
                rl = cl // N_CORES
                out[b * T + cs + c * rl: b * T + cs + (c + 1) * rl] = \
                    res[b * RPB + ro: b * RPB + ro + rl]
    out += np.asarray(bo, dtype=np.float32)[None, :]
    return out.reshape(B, T, C), r


def kernel(x, Wq, Wk, Wv, Wo, bo):
    out, _ = _run(x, Wq, Wk, Wv, Wo, bo, trace=False)
    return out



# revision 3
# speedup vs baseline: 173.5093x; 173.5093x over previous
"""GQA attention kernel for 8 Trainium2 NeuronCores (tensor-parallel over heads).

Self-contained: hardcodes shapes from the problem spec.
  x  [2, 1024, 4096]  Wq [4096, 4096]  Wk/Wv [4096, 1024]  Wo [4096, 4096]  bo [4096]
  32 q heads, 8 KV groups, head dim 128, RoPE theta 10000, causal softmax.

Sharding: core c owns KV group c and query heads 4c..4c+3.
  - xT fed replicated (no input collective); Wq/Wk/Wv column-sharded per
    head group (RoPE-permuted, scale folded into Wq).
  - QKV projection keeps q/k (plus a half-rotated copy for RoPE) and vT
    resident in SBUF; attention runs per (batch, local head).
  - Wo is ROW-sharded: each core projects its own heads' attention output
    into a full-width partial [1024, 4096] per batch, then a ReduceScatter
    per batch sums partials and hands core c rows c*128..c*128+128.
  - Host reassembles rows and adds bo.
"""

import numpy as np
import ml_dtypes

import concourse.bass as bass
import concourse.bass_isa as bass_isa
import concourse.mybir as mybir
import concourse.tile as tile
from concourse import bacc
from concourse import bass_utils

N_CORES = 8
B, T, C = 2, 1024, 4096
H, G, D = 32, 8, 128
REP = H // G             # q heads per KV group/core = 4
BT = B * T               # 2048
HD_SHARD = REP * D       # 512 q columns per core
ROPE_THETA = 10000.0
NCH = T // 128           # 8 s/t blocks per batch
OUT_ROWS = BT // N_CORES  # 256 output rows per core (128 per batch)
# per-batch ReduceScatter chunking: (row_start, row_len) of the partial
RS_CHUNKS = {0: [(0, 512), (512, 512)], 1: [(0, 512), (512, 512)]}

F32 = mybir.dt.float32
F16 = mybir.dt.float16

_CACHE = {}

# packed causal probs: row-block j holds cols t in [j*128, T)
_POFF = [0] * NCH
for _j in range(1, NCH):
    _POFF[_j] = _POFF[_j - 1] + (T - (_j - 1) * 128)
_PTOT = _POFF[-1] + (T - (NCH - 1) * 128)  # 4608


def _balanced_chunks(total, maxc=512, quantum=128):
    nblk = total // quantum
    n = -(-total // maxc)
    sizes = []
    for i in range(n):
        take = nblk // n + (1 if i < nblk % n else 0)
        sizes.append(take * quantum)
    return sizes


def _build_nc():
    nc = bacc.Bacc("TRN2", target_bir_lowering=False, debug=False, num_devices=N_CORES)

    # ---- I/O ----
    xT = nc.dram_tensor("xT", [C, BT], F16, kind="ExternalInput")
    # weights pre-shuffled on host to [p][chunk][ko][m] so DMA runs are 8KB
    wq = nc.dram_tensor("wq", [128, REP * (C // 128) * 128], F16,
                        kind="ExternalInput")
    wk = nc.dram_tensor("wk", [128, (C // 128) * D], F16, kind="ExternalInput")
    wv = nc.dram_tensor("wv", [128, (C // 128) * D], F16, kind="ExternalInput")
    wo = nc.dram_tensor("wo", [HD_SHARD, C], F16, kind="ExternalInput")
    cos2 = nc.dram_tensor("cos2", [D, T], F16, kind="ExternalInput")
    sinpm = nc.dram_tensor("sinpm", [D, T], F16, kind="ExternalInput")
    tri = nc.dram_tensor("tri", [128, 128], F16, kind="ExternalInput")
    idin = nc.dram_tensor("idin", [128, 128], F16, kind="ExternalInput")
    out = nc.dram_tensor("out", [OUT_ROWS, C], F16, kind="ExternalOutput")

    # ---- DRAM scratch ----
    partial = [
        nc.dram_tensor(f"partial_{b}", [T, C], F16, kind="Internal")
        for b in range(B)
    ]
    rsout = [
        nc.dram_tensor(f"rsout_{b}", [T // N_CORES, C], F16, kind="Internal")
        for b in range(B)
    ]

    M_ALL = HD_SHARD + 2 * D   # 768
    KSUB = C // 128            # 32  (contraction chunks)
    NT = 512                   # stage-1 t-cols per chunk
    NNT = BT // NT             # 4
    # m-chunk order: two passes per n-chunk; k & v first so attention can start
    M_GROUPS = [[4, 5, 0], [1, 2, 3]]

    xT_r = xT[:].rearrange("(ko p) t -> p ko t", p=128)
    wq_r = wq[:].rearrange("p (h ko m) -> p h ko m", h=REP, ko=KSUB)
    wk_r = wk[:].rearrange("p (ko m) -> p ko m", ko=KSUB)
    wv_r = wv[:].rearrange("p (ko m) -> p ko m", ko=KSUB)
    wo_r = wo[:].rearrange("(h p) co -> p h co", p=128)

    from contextlib import ExitStack
    with tile.TileContext(nc) as tc:
        with ExitStack() as _stk:
            _p = lambda **kw: _stk.enter_context(tc.tile_pool(**kw))
            consts = _p(name="consts", bufs=1)
            persist = _p(name="persist", bufs=1)
            s2_probs = _p(name="s2_probs", bufs=2)
            s2_attnT = _p(name="s2_attnT", bufs=8)
            s2_kv = _p(name="s2_kv", bufs=2)
            s2_q = _p(name="s2_q", bufs=2)
            s2_tmp = _p(name="s2_tmp", bufs=2)
            s2_misc = _p(name="s2_misc", bufs=2)
            s2_dseg = _p(name="s2_dseg", bufs=2)
            s2_ps_sc = _p(name="s2_ps_sc", bufs=2, space="PSUM")
            s2_ps_pv = _p(name="s2_ps_pv", bufs=2, space="PSUM")
            s2_ps_tr = _p(name="s2_ps_tr", bufs=1, space="PSUM")
            # ---------- constants (loads emitted after first weights) ----------
            ident = consts.tile([128, 128], F16)
            tri_sb = consts.tile([128, 128], F16)
            cos_sb = consts.tile([128, T], F16)
            sin_sb = consts.tile([128, T], F16)

            # ---------- persistent stage-1 outputs ----------
            qkT = persist.tile([128, REP + 1, BT], F16)      # q0..q3, k
            qkTs = persist.tile([128, REP + 1, BT], F16)     # half-rotated copy
            vT = persist.tile([128, BT], F16)
            wo_sb = persist.tile([128, REP, C], F16)

            # ================= Stage 1: QKV projection =================
            with ExitStack() as _stk1:
                _p1 = lambda **kw: _stk1.enter_context(tc.tile_pool(**kw))
                s1_w = _p1(name="s1_w", bufs=1)
                s1_x = _p1(name="s1_x", bufs=2)
                s1_psum = _p1(name="s1_psum", bufs=3, space="PSUM")
                # [p, m-chunk(6), ko, 128]: q0..q3, k, v
                w_sb = s1_w.tile([128, 6, KSUB, 128], F16)
                nc.scalar.dma_start(w_sb[:, 4], wk_r)
                nc.scalar.dma_start(w_sb[:, 5], wv_r)
                nc.scalar.dma_start(w_sb[:, 0], wq_r[:, 0])

                def _late_weights(kq):
                    if kq in (1, 3, 5):
                        hh = (kq + 1) // 2
                        nc.scalar.dma_start(w_sb[:, hh], wq_r[:, hh])
                    elif kq == 7:
                        # consts aren't needed until RoPE (~170us in)
                        nc.scalar.dma_start(ident[:], idin[:])
                        nc.scalar.dma_start(tri_sb[:], tri[:])
                        nc.scalar.dma_start(cos_sb[:], cos2[:])
                        nc.scalar.dma_start(sin_sb[:], sinpm[:])

                # PE warmup: dummy matmuls on memset tiles while the first
                # weight/x DMAs are in flight, so the HAM clock gate opens
                # (1.2 -> 2.4 GHz) before real work arrives
                wu_st = s1_w.tile([128, 128], F16, name="wu_st", tag="wu_st")
                nc.vector.memset(wu_st[:], 0.0)
                wu_mv = s1_w.tile([128, 512], F16, name="wu_mv", tag="wu_mv")
                nc.vector.memset(wu_mv[:], 0.0)
                wu_ps = s1_psum.tile([128, NT], F32, name="wu_ps", tag="s1ps")
                for _wu in range(12):
                    nc.tensor.matmul(
                        wu_ps[:], wu_st[:], wu_mv[:],
                        start=(_wu == 0), stop=(_wu == 11),
                    )

                first_grp_done = False
                for n in range(NNT):
                    nsl = slice(n * NT, (n + 1) * NT)
                    for grp in M_GROUPS:
                        psums = {
                            m: s1_psum.tile([128, NT], F32,
                                            name=f"s1ps_{m}", tag="s1ps")
                            for m in grp
                        }
                        for kq in range(KSUB // 4):  # 8 loads of 4 k-slices
                            x_sb = s1_x.tile([128, 4, NT], F16, tag="s1x")
                            nc.sync.dma_start(
                                x_sb[:], xT_r[:, kq * 4:(kq + 1) * 4, nsl]
                            )
                            if not first_grp_done:
                                _late_weights(kq)
                            for m in grp:
                                for k in range(4):
                                    nc.tensor.matmul(
                                        psums[m][:],
                                        w_sb[:, m, kq * 4 + k, :],
                                        x_sb[:, k, :],
                                        start=(kq == 0 and k == 0),
                                        stop=(kq == KSUB // 4 - 1 and k == 3),
                                    )
                        first_grp_done = True
                        for m in grp:
                            if m == 5:
                                nc.vector.tensor_copy(vT[:, nsl], psums[m][:])
                            else:
                                nc.vector.tensor_copy(qkT[:, m, nsl], psums[m][:])
                                # half-rotated copy for RoPE (SBUF->SBUF DMA,
                                # ACT queue to keep x prefetch flowing on SP)
                                nc.scalar.dma_start(
                                    qkTs[0:64, m, nsl], qkT[64:128, m, nsl])
                                nc.scalar.dma_start(
                                    qkTs[64:128, m, nsl], qkT[0:64, m, nsl])
                if True:
                    nc.scalar.dma_start(wo_sb[:], wo_r)

            # ================= Stage 2 + 3 per batch =================
            with ExitStack() as _stk3:
                _p3 = lambda **kw: _stk3.enter_context(tc.tile_pool(**kw))
                s3_ev = _p3(name="s3_ev", bufs=3)
                s3_psum = _p3(name="s3_psum", bufs=2, space="PSUM")

                def rope(dst, m, tcol):
                    tmp = s2_tmp.tile([128, T], F16, tag="rope_tmp")
                    nc.vector.tensor_tensor(
                        tmp[:], qkTs[:, m, tcol], sin_sb[:], mybir.AluOpType.mult)
                    nc.vector.tensor_tensor(
                        dst[:], qkT[:, m, tcol], cos_sb[:], mybir.AluOpType.mult)
                    nc.vector.tensor_tensor(
                        dst[:], dst[:], tmp[:], mybir.AluOpType.add)

                pending_rs = []

                def _flush_rs(pb):
                    if no_rs:
                        return
                    for (cs, cl) in RS_CHUNKS[pb]:
                        ro = cs // N_CORES
                        rl = cl // N_CORES
                        nc.gpsimd.collective_compute(
                            "ReduceScatter",
                            mybir.AluOpType.add,
                            replica_groups=[list(range(N_CORES))],
                            ins=[partial[pb][cs:cs + cl, :].opt()],
                            outs=[rsout[pb][ro:ro + rl, :].opt()],
                        )
                        nc.scalar.dma_start(
                            out[pb * (T // N_CORES) + ro:
                                pb * (T // N_CORES) + ro + rl, :],
                            rsout[pb][ro:ro + rl, :],
                        )

                for b in range(B):
                    tcol = slice(b * T, (b + 1) * T)
                    k_rope = s2_kv.tile([128, T], F16, tag="k_rope")
                    rope(k_rope, REP, tcol)
                    # v: transpose vT blocks to [s, d]
                    v_sb = s2_kv.tile([128, NCH, D], F16, tag="v_sb")
                    for j in range(NCH):
                        ps_tr = s2_ps_tr.tile([128, 128], F16, tag="ps_tr")
                        nc.tensor.transpose(
                            ps_tr[:], vT[:, b * T + j * 128:b * T + (j + 1) * 128],
                            ident[:])
                        nc.vector.tensor_copy(v_sb[:, j, :], ps_tr[:])

                    attnTs = []
                    for h in range(REP):
                        q_rope = s2_q.tile([128, T], F16, tag="q_rope")
                        rope(q_rope, h, tcol)

                        # scoresT[s, t] = k_rope.T @ q_rope -> exp -> probs
                        probs = s2_probs.tile([128, _PTOT], F16, tag="probs")
                        acc = s2_misc.tile([1, T], F16, tag="acc")
                        for j in range(NCH):
                            t0 = j * 128
                            seg = T - t0
                            off = t0
                            for w in _balanced_chunks(seg):
                                ps_sc = s2_ps_sc.tile([128, 512], F32, tag="ps_sc")
                                nc.tensor.matmul(
                                    ps_sc[:, 0:w],
                                    k_rope[:, t0:t0 + 128],
                                    q_rope[:, off:off + w],
                                    start=True,
                                    stop=True,
                                )
                                nc.scalar.activation(
                                    probs[:, _POFF[j] + off - t0:
                                          _POFF[j] + off - t0 + w],
                                    ps_sc[:, 0:w],
                                    mybir.ActivationFunctionType.Exp,
                                )
                                off += w
                            # mask on gpsimd: feeds the partition reduce on
                            # the same queue (no cross-engine sem hop)
                            nc.gpsimd.tensor_tensor(
                                probs[:, _POFF[j]:_POFF[j] + 128],
                                probs[:, _POFF[j]:_POFF[j] + 128],
                                tri_sb[:],
                                mybir.AluOpType.mult,
                            )
                            # denominator: partial column-sums of this segment
                            dseg = s2_dseg.tile([128, T], F16, tag="dseg")
                            nc.gpsimd.partition_all_reduce(
                                dseg[:, 0:seg],
                                probs[:, _POFF[j]:_POFF[j] + seg],
                                channels=128,
                                reduce_op=bass_isa.ReduceOp.add,
                            )
                            with nc.allow_low_precision(
                                    reason="f16 softmax denom, 0.1% rel"):
                                if j == 0:
                                    nc.vector.tensor_copy(
                                        acc[:, :], dseg[0:1, 0:seg])
                                else:
                                    nc.vector.tensor_tensor(
                                        acc[:, t0:T], acc[:, t0:T],
                                        dseg[0:1, 0:seg],
                                        mybir.AluOpType.add)
                        rcp1 = s2_misc.tile([1, T], F16, tag="rcp1")
                        with nc.allow_low_precision(
                                reason="f16 softmax denom, 0.1% rel"):
                            nc.vector.reciprocal(rcp1[:], acc[:])
                        recipB = s2_misc.tile([128, T], F16, tag="recipB")
                        nc.gpsimd.partition_broadcast(recipB[:], rcp1[:])

                        # PV: attnT[d, t] = sum_j v_j.T @ probs_j, then
                        # normalize by the softmax denominator on eviction
                        attnT = s2_attnT.tile([128, NCH, 128], F16, tag="attnT")
                        attnTs.append(attnT)
                        NR = 512
                        for r in range(T // NR):
                            ps_pv = s2_ps_pv.tile([128, NR], F32, tag="ps_pv")
                            jmax = (r * NR) // 128 + NR // 128 - 1
                            for j in range(jmax + 1):
                                a = max(0, j * 128 - r * NR)
                                w = NR - a
                                tg = r * NR + a  # global t of slice start
                                nc.tensor.matmul(
                                    ps_pv[:, a:NR],
                                    v_sb[:, j, :],
                                    probs[:, _POFF[j] + tg - j * 128:
                                          _POFF[j] + tg - j * 128 + w],
                                    start=(j == 0),
                                    stop=(j == jmax),
                                )
                            nc.vector.tensor_tensor(
                                attnT[:, r * NR // 128:(r + 1) * NR // 128, :],
                                ps_pv[:],
                                recipB[:, r * NR:(r + 1) * NR],
                                mybir.AluOpType.mult,
                            )

                    # ---- Stage 3: partial = attn_local @ Wo_local ----
                    for i in range(NCH):
                        for cc2 in range(C // 1024):
                            ev3 = s3_ev.tile([128, 1024], F16, tag="s3ev")
                            for sub in range(2):
                                cch = cc2 * 2 + sub
                                ps3 = s3_psum.tile([128, 512], F32, tag="s3ps")
                                for h in range(REP):
                                    nc.tensor.matmul(
                                        ps3[:],
                                        attnTs[h][:, i, :],
                                        wo_sb[:, h, cch * 512:(cch + 1) * 512],
                                        start=(h == 0),
                                        stop=(h == REP - 1),
                                    )
                                nc.vector.tensor_copy(
                                    ev3[:, sub * 512:(sub + 1) * 512], ps3[:])
                            nc.sync.dma_start(
                                partial[b][i * 128:(i + 1) * 128,
                                           cc2 * 1024:(cc2 + 1) * 1024],
                                ev3[:],
                            )

                    # ---- ReduceScatter emission is deferred so its queue
                    # wait never blocks later gpsimd work (see _flush_rs) ----
                    pending_rs.append(b)
                    if b == B - 1:
                        for pb in pending_rs:
                            _flush_rs(pb)
                        pending_rs.clear()

    nc.compile()
    return nc


def _host_prep(x, Wq, Wk, Wv, Wo, bo):
    x = np.asarray(x, dtype=np.float32)
    Wq = np.asarray(Wq, dtype=np.float32)
    Wk = np.asarray(Wk, dtype=np.float32)
    Wv = np.asarray(Wv, dtype=np.float32)
    Wo = np.asarray(Wo, dtype=np.float32)

    xT = np.ascontiguousarray(x.reshape(BT, C).astype(np.float16).T)

    scale = np.float32(D ** -0.5)
    # rope-permute ([evens | odds] within each head) via reshape+transpose
    Wqp = np.ascontiguousarray(
        (Wq * scale).astype(np.float16)
        .reshape(C, H, D // 2, 2).transpose(0, 1, 3, 2).reshape(C, H, D))
    Wkp = np.ascontiguousarray(
        Wk.astype(np.float16)
        .reshape(C, G, D // 2, 2).transpose(0, 1, 3, 2).reshape(C, G, D))
    Wv16 = Wv.astype(np.float16).reshape(C, G, D)
    Wo16 = Wo.astype(np.float16)

    freqs = 1.0 / (ROPE_THETA ** (np.arange(0, D, 2, dtype=np.float64) / D))
    angle = np.arange(T, dtype=np.float64)[:, None] * freqs[None, :]  # [T, 64]
    cosh = np.cos(angle).T.astype(np.float16)   # [64, T]
    sinh = np.sin(angle).T.astype(np.float16)
    cos2 = np.ascontiguousarray(np.vstack([cosh, cosh]))       # [128, T]
    sinpm = np.ascontiguousarray(np.vstack([-sinh, sinh]))     # [128, T]

    sidx = np.arange(128)[:, None]
    tidx = np.arange(128)[None, :]
    tri = np.ascontiguousarray((sidx <= tidx).astype(np.float16))
    ident = np.eye(128, dtype=np.float16)
    perm = np.zeros((128, 128), dtype=np.float16)
    perm[(np.arange(128) + 64) % 128, np.arange(128)] = 1.0

    def _pshuf(W):
        # [C, M] -> [p, M/128 chunks? no: [p][ko][m]] flattened per partition
        KO = C // 128
        M = W.shape[1]
        return np.ascontiguousarray(
            W.reshape(KO, 128, M).transpose(1, 0, 2).reshape(128, KO * M))

    in_maps = []
    for c in range(N_CORES):
        Wqc = Wqp[:, c * REP:(c + 1) * REP]  # [C, REP, D]
        # [p][h][ko][m]
        wq_host = np.ascontiguousarray(
            Wqc.reshape(C // 128, 128, REP, D)
            .transpose(1, 2, 0, 3).reshape(128, REP * (C // 128) * D))
        in_maps.append({
            "xT": xT,
            "wq": wq_host,
            "wk": _pshuf(Wkp[:, c]),
            "wv": _pshuf(Wv16[:, c]),
            "wo": np.ascontiguousarray(Wo16[c * HD_SHARD:(c + 1) * HD_SHARD, :]),
            "cos2": cos2,
            "sinpm": sinpm,
            "tri": tri,
            "idin": ident,
            "permin": perm,
        })
    return in_maps


def _run(x, Wq, Wk, Wv, Wo, bo, trace=False, trace_cores=None):
    in_maps = _host_prep(x, Wq, Wk, Wv, Wo, bo)
    if "nc" not in _CACHE:
        _CACHE["nc"] = _build_nc()
    nc = _CACHE["nc"]
    r = bass_utils.run_bass_kernel_spmd(
        nc, in_maps, core_ids=list(range(N_CORES)),
        trace=trace, trace_cores=trace_cores,
    )
    out = np.empty((BT, C), dtype=np.float32)
    for c in range(N_CORES):
        out[c * OUT_ROWS:(c + 1) * OUT_ROWS] = r.results[c]["out"]
    out += np.asarray(bo, dtype=np.float32)[None, :]
    return out.reshape(B, T, C), r


def kernel(x, Wq, Wk, Wv, Wo, bo):
    out, _ = _run(x, Wq, Wk, Wv, Wo, bo, trace=False)
    return out

